# revision 2
# baseline (speedup 1.0000x reference)
"""Swin-style window attention kernel for 8 TRN2 NeuronCores (SPMD, batch-sharded).

v2 layout strategy per core (16 windows):
  - all inputs bf16 (host-cast); exp(bias) precomputed on host.
  - xT via PE transpose (bf16, 1 cyc/row); qkv projection bf16 (N=392).
  - q,k feature-major [d,tok] bf16; v token-major [tok,(h,d)] bf16.
  - attention in 2-head sub-groups with double-buffered QK psum:
    QK^T transposed (attnT [k,196]) 32-row-packed matmuls, exp on ACT,
    * exp(bias) on DVE (bf16 2x), AV dense + replicated denominators via
    ones-matmuls (4-head psum bank), reciprocal + normalize on DVE.
  - PE emission staggered: QK(sg+1) issued before AV(sg) to hide the
    exp->mul chain latency.
  - proj bf16, bias added during psum evacuation (DVE), y stores via SP
    DMA with explicit x prefetch ordering (Pool left idle; no memsets in
    the steady loop, no qk zero-padding).
"""
import numpy as np

B, NT, CH = 128, 196, 512
H, D = 16, 32
NCORES = 8
WPC = B // NCORES  # windows per core

_CACHE = {}


def _build():
    import concourse.bass as bass
    import concourse.mybir as mybir
    import concourse.tile as tile
    from concourse import bacc

    fp32 = mybir.dt.float32
    bf16 = mybir.dt.bfloat16
    fp16 = mybir.dt.float16
    f8 = mybir.dt.float8e4
    DR = mybir.MatmulPerfMode.DoubleRow
    AF = mybir.ActivationFunctionType

    nc = bacc.Bacc("TRN2", target_bir_lowering=False, debug=False, num_devices=NCORES)

    # x pre-transposed on host: [pair, ki, ko, wi*196+q] = x[w, q, 128*ko+ki]
    # fp8 copy feeds the q,k projection (DoubleRow); bf16 copy feeds v.
    x8d = nc.dram_tensor("x8", [WPC // 2, 128, 4, 2 * NT], f8, kind="ExternalInput")
    x = nc.dram_tensor("x", [WPC // 2, 128, 4, 2 * NT], bf16, kind="ExternalInput")
    wqk_d = nc.dram_tensor("wqk", [CH, 2 * CH], f8, kind="ExternalInput")
    wv_d = nc.dram_tensor("wv", [CH, CH], bf16, kind="ExternalInput")
    ebt_d = nc.dram_tensor("ebt", [128, H, 2 * NT], bf16, kind="ExternalInput")
    proj_w = nc.dram_tensor("proj_w", [CH, CH], bf16, kind="ExternalInput")
    proj_b = nc.dram_tensor("proj_b", [1, CH], fp32, kind="ExternalInput")
    y = nc.dram_tensor("y", [WPC, NT, CH], fp32, kind="ExternalOutput")

    with tile.TileContext(nc) as tc:
        with (
            tc.tile_pool(name="const", bufs=1) as cpool,
            tc.tile_pool(name="work", bufs=2) as wpool,
            tc.tile_pool(name="attn", bufs=3) as apool,
            tc.tile_pool(name="ps_qk", bufs=2, space="PSUM") as ps_qk,
            tc.tile_pool(name="ps_av", bufs=2, space="PSUM") as ps_av,
            tc.tile_pool(name="ps_ms", bufs=2, space="PSUM") as ps_ms,
        ):
            # ---------------- one-time setup ----------------
            # persistent fp8 q,k feature tiles (ping-pong across pairs).
            # Slot a=1 stays all-zero: it is the second k-tile of every
            # DoubleRow QK^T matmul (the PE streams 2 tiles/cycle in DR mode,
            # so a 32-deep contraction runs at 0.5 cyc/row with a zero mate).
            # First tile zeroed on idle DVE right away (needed ~4us in);
            # second on Pool (not needed until pair 1).
            qk8s = []
            for pp_ in range(2):
                qk8t = cpool.tile([128, 2, 8, 2 * NT], f8, tag=f"qk8_{pp_}",
                                  name=f"qk8_{pp_}")
                if pp_ == 0:
                    nc.vector.memset(qk8t[:, 1, :, :], 0.0)
                else:
                    nc.gpsimd.memset(qk8t[:, 1, :, :], 0.0)
                qk8s.append(qk8t)

            # x loads own the SP queue; weights go via ACT hwdge (in
            # first-needed order); small/late tensors via gpsimd swdge.
            def load_x(pair):
                x8 = wpool.tile([128, 4, 2 * NT], f8, tag=f"x8{pair % 2}")
                nc.sync.dma_start(x8[:], x8d.ap()[pair])
                xT = wpool.tile([128, 4, 2 * NT], bf16, tag=f"xT{pair % 2}")
                nc.sync.dma_start(xT[:], x.ap()[pair])
                return xT, x8

            xs_cur = load_x(0)

            wqk = cpool.tile([128, 4, 2 * CH], f8, tag="wqk")
            nc.scalar.dma_start(wqk[:], wqk_d.ap().rearrange("(ko ki) m -> ki ko m", ki=128))
            wv = cpool.tile([128, 4, CH], bf16, tag="wv")
            nc.scalar.dma_start(wv[:], wv_d.ap().rearrange("(ko ki) m -> ki ko m", ki=128))
            # ebt in 4-head chunks so the big table transfer never delays the
            # startup-critical x/weight DMAs; chunk g unblocks groups in order
            ebt = cpool.tile([128, H, 2 * NT], bf16, tag="ebt")
            for g4 in range(4):
                nc.gpsimd.dma_start(ebt[:, 4 * g4:4 * g4 + 4, :],
                                    ebt_d.ap()[:, 4 * g4:4 * g4 + 4, :])
            pw = cpool.tile([128, 4, CH], bf16, tag="pw")
            nc.scalar.dma_start(pw[:], proj_w.ap().rearrange("(ko ki) m -> ki ko m", ki=128))

            b_row = cpool.tile([1, CH], fp32, tag="brow")
            nc.gpsimd.dma_start(b_row[:], proj_b.ap())
            b_bcast = cpool.tile([128, CH], fp32, tag="bb")
            nc.gpsimd.partition_broadcast(b_bcast[:], b_row[:], channels=128)

            ones32 = cpool.tile([128, 32], bf16, tag="ones")
            nc.gpsimd.memset(ones32[:], 1.0)

            pending = []  # deferred proj emission (fills next pair's qkv stage)

            def emit_qk(qk8, wi, sg):
                wo = wi * NT
                qkps = ps_qk.tile([128, 2, 512], fp32, tag="qkps")
                for j in range(2):
                    h = 2 * sg + j
                    hb = 32 * (h % 4)
                    qblk, kblk = h // 4, 4 + h // 4
                    rhs_q = qk8[hb:hb + 32, :, qblk, wo:wo + NT]
                    nc.tensor.matmul(qkps[:, j, 0:NT],
                                     qk8[hb:hb + 32, :, kblk, wo:wo + 128],
                                     rhs_q, start=True, stop=True,
                                     tile_position=(hb, 0), perf_mode=DR)
                    if wi == 0:
                        nc.tensor.matmul(qkps[:, j, NT:2 * NT],
                                         qk8[hb:hb + 32, :, kblk, 128:256],
                                         rhs_q, start=True, stop=True,
                                         tile_position=(hb, 0), perf_mode=DR)
                    else:
                        nc.tensor.matmul(qkps[0:68, j, NT:2 * NT],
                                         qk8[hb:hb + 32, :, kblk, wo + 128:wo + NT],
                                         rhs_q, start=True, stop=True,
                                         tile_position=(hb, 0), perf_mode=DR)
                return qkps

            def attn_window(qk8, v_sb, wi, qlist):
                # qlist: already-emitted qkps tiles (pipeline pre-fill)
                P = len(qlist)
                # one tile per 4-head group so proj block bl only depends on
                # its own group's normalize (not the last one)
                attn_rs = [apool.tile([128, NT], bf16, tag=f"attn_r{g}",
                                      name=f"attn_r{g}")
                           for g in range(4)]
                avps = None
                for sg in range(8):
                    qkps = qlist[sg]
                    # exp (ACT) then *exp(bias) (DVE, bf16 2x); the softmax
                    # 1/sqrt(d) rides along as the activation pre-scale
                    esb = apool.tile([128, 2, 2 * NT], bf16, tag="esb")
                    nc.scalar.activation(esb[:], qkps[:, :, 0:2 * NT], AF.Exp,
                                         scale=0.17677669529663687)
                    et = apool.tile([128, 2, 2 * NT], bf16, tag="et")
                    nc.vector.tensor_mul(et[:], esb[:], ebt[:, 2 * sg:2 * sg + 2, :])
                    if sg + P < 8:
                        qlist.append(emit_qk(qk8, wi, sg + P))
                    # AV + replicated denominators into 4-head psum bank
                    if sg % 2 == 0:
                        avps = ps_av.tile([128, 512], fp32, tag="avps")
                    for j in range(2):
                        h = 2 * sg + j
                        band = 32 * (h % 4)
                        nc.tensor.matmul(avps[band:band + 32, 0:NT],
                                         v_sb[:, 0, h, :], et[:, j, 0:NT],
                                         start=True, stop=False,
                                         tile_position=(0, band))
                        nc.tensor.matmul(avps[band:band + 32, 0:NT],
                                         v_sb[0:68, 1, h, :], et[0:68, j, NT:2 * NT],
                                         start=False, stop=True,
                                         tile_position=(0, band))
                        nc.tensor.matmul(avps[band:band + 32, 256:256 + NT],
                                         ones32[:], et[:, j, 0:NT],
                                         start=True, stop=False,
                                         tile_position=(0, band))
                        nc.tensor.matmul(avps[band:band + 32, 256:256 + NT],
                                         ones32[0:68, :], et[0:68, j, NT:2 * NT],
                                         start=False, stop=True,
                                         tile_position=(0, band))
                    if sg % 2 == 1:
                        g = sg // 2
                        r_d = apool.tile([128, NT], fp16, tag="rd")
                        with nc.allow_low_precision(reason="softmax recip in fp16"):
                            nc.vector.reciprocal(r_d[:], avps[:, 256:256 + NT])
                        nc.vector.tensor_mul(attn_rs[g][:], avps[:, 0:NT], r_d[:])
                return attn_rs

            def proj_emit(w, wi, attn_rs):
                # deferred (w1) proj borrows the attention psum pool: during
                # the next pair's qkv stage it is idle, so its banks don't
                # steal the qpv rotation from under the matmuls
                for tch, tsz in ((0, 128), (1, 68)):
                    if wi == 0:
                        pp = ps_ms.tile([128, 512], fp32, tag="ms", name="pp")
                    else:
                        pp = ps_av.tile([128, 512], fp32, tag="avps", name="ppd")
                    for bl in range(4):
                        nc.tensor.matmul(pp[0:tsz, 0:CH],
                                         attn_rs[bl][:, tch * 128:tch * 128 + tsz],
                                         pw[:, bl, :], start=(bl == 0), stop=(bl == 3))
                    yt = wpool.tile([128, CH], fp32, tag=f"yt{wi}{tch}")
                    nc.vector.tensor_add(yt[0:tsz, :], pp[0:tsz, 0:CH], b_bcast[0:tsz, :])
                    nc.sync.dma_start(y.ap()[w, tch * 128:tch * 128 + tsz, :], yt[0:tsz, :])

            # ---------------- main loop ----------------
            for pair in range(WPC // 2):
                # prefetch next pair's x before anything else hits SP's queue
                xs_next = load_x(pair + 1) if pair + 1 < WPC // 2 else None
                xT, x8 = xs_cur

                # q,k feature-major fp8 [128, slot, blk, 392] (fp8 DoubleRow
                # matmuls); softmax 1/sqrt(d) is applied later in the exp.
                qk8 = qk8s[pair % 2]
                q0 = []
                for mb in range(8):
                    qpv = ps_ms.tile([128, 512], fp32, tag="ms", name="qpv")
                    for t in range(2):
                        nc.tensor.matmul(qpv[:, 0:2 * NT],
                                         wqk[:, 2 * t:2 * t + 2, mb * 128:(mb + 1) * 128],
                                         x8[:, 2 * t:2 * t + 2, :],
                                         start=(t == 0), stop=(t == 1), perf_mode=DR)
                    # undo the host fp8 weight pre-scale (x64). Evacs alternate
                    # ACT/DVE: the DR matmuls are so cheap the stage is
                    # evac-paced, so one engine alone would bottleneck it.
                    if mb % 2 == 0:
                        nc.scalar.activation(qk8[:, 0, mb, :], qpv[:, 0:2 * NT],
                                             AF.Copy, scale=1.0 / 64.0)
                    else:
                        nc.vector.tensor_scalar_mul(qk8[:, 0, mb, :], qpv[:, 0:2 * NT],
                                                    1.0 / 64.0)
                    if mb == 2 and pending:
                        pending.pop()()  # prev pair's w1 proj fills this stage
                    if mb == 4:
                        # w0's first two QK sub-groups only need feature
                        # blocks 0 and 4 — start them mid-stage so the exp
                        # chain drains while the rest of qkv runs
                        q0.append(emit_qk(qk8, 0, 0))
                    elif mb == 5:
                        q0.append(emit_qk(qk8, 0, 1))

                # v token-major [128(tok), 2(chunk), H, D] bf16, per window
                vs = [None, None]
                for wi in range(2):
                    wo = wi * NT
                    v_sb = wpool.tile([128, 2, H, D], bf16, tag=f"v{wi}")
                    vs[wi] = v_sb
                    for tch, tsz in ((0, 128), (1, 68)):
                        vpv = ps_ms.tile([128, 512], fp32, tag="ms", name="vpv")
                        for kc in range(4):
                            nc.tensor.matmul(
                                vpv[0:tsz, 0:CH],
                                xT[:, kc, wo + tch * 128: wo + tch * 128 + tsz],
                                wv[:, kc, :],
                                start=(kc == 0), stop=(kc == 3))
                        # split the two chunk evacs across ACT/Pool so v_sb is
                        # ready before the first AV matmul needs it and DVE
                        # stays free for the softmax stream
                        if tch == 0:
                            nc.scalar.activation(
                                v_sb[0:tsz, tch, :, :].rearrange("p h d -> p (h d)"),
                                vpv[0:tsz, 0:CH], AF.Copy)
                        else:
                            nc.vector.tensor_copy(
                                v_sb[0:tsz, tch, :, :].rearrange("p h d -> p (h d)"),
                                vpv[0:tsz, 0:CH])

                attn_r0 = attn_window(qk8, vs[0], 0, q0)
                q1 = [emit_qk(qk8, 1, 0)]
                proj_emit(2 * pair, 0, attn_r0)
                attn_r1 = attn_window(qk8, vs[1], 1, q1)
                if pair == WPC // 2 - 1:
                    proj_emit(2 * pair + 1, 1, attn_r1)
                else:
                    pending.append(
                        lambda w=2 * pair + 1, a=attn_r1: proj_emit(w, 1, a))

                xs_cur = xs_next

            while pending:
                pending.pop()()

    nc.compile()
    return nc


def _prep_ebt(rel_pos_index, rel_bias_table):
    # ebt[p, h, khi*196 + q] = exp(table[idx[q, p + 128*khi], h]) (1.0 where k pad)
    idx = np.asarray(rel_pos_index).astype(np.int64)
    table = np.asarray(rel_bias_table, dtype=np.float32)
    g = table[idx]                      # [q, k, H]
    out = np.zeros((256, H, NT), dtype=np.float32)
    out[:NT] = g.transpose(1, 2, 0)     # [k, H, q]
    out = np.exp(out)
    return np.ascontiguousarray(
        out.reshape(2, 128, H, NT).transpose(1, 2, 0, 3).reshape(128, H, 2 * NT))


def kernel(x, qkv_w, rel_bias_table, proj_w, proj_b, rel_pos_index):
    import ml_dtypes
    from concourse.bass_utils import run_bass_kernel_spmd

    if "nc" not in _CACHE:
        _CACHE["nc"] = _build()
    nc = _CACHE["nc"]

    bf16 = ml_dtypes.bfloat16
    f8 = ml_dtypes.float8_e4m3
    # host pre-transpose: [b/2, ki, ko, wi*196+q] = x[w, q, 128*ko+ki]
    xf = np.asarray(x, dtype=np.float32).reshape(B // 2, 2, NT, 4, 128)
    xf = np.ascontiguousarray(xf.transpose(0, 4, 3, 1, 2).reshape(B // 2, 128, 4, 2 * NT))
    x16 = xf.astype(bf16)
    x8 = xf.astype(f8)
    qkv_f = np.asarray(qkv_w, dtype=np.float32)
    # x64 pre-scale lifts the tiny weights out of fp8 subnormal range;
    # undone (with the softmax scale for q) in the on-device evacuation.
    wqk = np.ascontiguousarray(qkv_f[:, :2 * CH] * 64.0).astype(f8)
    wv = np.ascontiguousarray(qkv_f[:, 2 * CH:]).astype(bf16)
    ebt = _prep_ebt(rel_pos_index, rel_bias_table).astype(bf16)
    pw = np.ascontiguousarray(np.asarray(proj_w, dtype=np.float32)).astype(bf16)
    pb = np.ascontiguousarray(np.asarray(proj_b), dtype=np.float32).reshape(1, CH)

    hw = WPC // 2
    in_maps = []
    for c in range(NCORES):
        in_maps.append({
            "x": x16[c * hw:(c + 1) * hw],
            "x8": x8[c * hw:(c + 1) * hw],
            "wqk": wqk,
            "wv": wv,
            "ebt": ebt,
            "proj_w": pw,
            "proj_b": pb,
        })
    res = run_bass_kernel_spmd(nc, in_maps, core_ids=list(range(NCORES)))
    out = np.concatenate([r["y"] for r in res.results], axis=0)
    return out.astype(np.float32)


if __name__ == "__main__":
    pass


# revision 3
# speedup vs baseline: 1.0004x; 1.0004x over previous
"""Swin-style window attention kernel for 8 TRN2 NeuronCores (SPMD, batch-sharded).

v2 layout strategy per core (16 windows):
  - all inputs bf16 (host-cast); exp(bias) precomputed on host.
  - xT via PE transpose (bf16, 1 cyc/row); qkv projection bf16 (N=392).
  - q,k feature-major [d,tok] bf16; v token-major [tok,(h,d)] bf16.
  - attention in 2-head sub-groups with double-buffered QK psum:
    QK^T transposed (attnT [k,196]) 32-row-packed matmuls, exp on ACT,
    * exp(bias) on DVE (bf16 2x), AV dense + replicated denominators via
    ones-matmuls (4-head psum bank), reciprocal + normalize on DVE.
  - PE emission staggered: QK(sg+1) issued before AV(sg) to hide the
    exp->mul chain latency.
  - proj bf16, bias added during psum evacuation (DVE), y stores via SP
    DMA with explicit x prefetch ordering (Pool left idle; no memsets in
    the steady loop, no qk zero-padding).
"""
import numpy as np

B, NT, CH = 128, 196, 512
H, D = 16, 32
NCORES = 8
WPC = B // NCORES  # windows per core

_CACHE = {}


def _build():
    import concourse.bass as bass
    import concourse.mybir as mybir
    import concourse.tile as tile
    from concourse import bacc

    fp32 = mybir.dt.float32
    bf16 = mybir.dt.bfloat16
    fp16 = mybir.dt.float16
    f8 = mybir.dt.float8e4
    DR = mybir.MatmulPerfMode.DoubleRow
    AF = mybir.ActivationFunctionType

    nc = bacc.Bacc("TRN2", target_bir_lowering=False, debug=False, num_devices=NCORES)

    # x pre-transposed on host: [pair, ki, ko, wi*196+q] = x[w, q, 128*ko+ki]
    # fp8 copy feeds the q,k projection (DoubleRow); bf16 copy feeds v.
    x8d = nc.dram_tensor("x8", [WPC // 2, 128, 4, 2 * NT], f8, kind="ExternalInput")
    x = nc.dram_tensor("x", [WPC // 2, 128, 4, 2 * NT], bf16, kind="ExternalInput")
    wqk_d = nc.dram_tensor("wqk", [CH, 2 * CH], f8, kind="ExternalInput")
    wv_d = nc.dram_tensor("wv", [CH, CH], bf16, kind="ExternalInput")
    ebt_d = nc.dram_tensor("ebt", [128, H, 2 * NT], bf16, kind="ExternalInput")
    proj_w = nc.dram_tensor("proj_w", [CH, CH], bf16, kind="ExternalInput")
    proj_b = nc.dram_tensor("proj_b", [1, CH], fp32, kind="ExternalInput")
    y = nc.dram_tensor("y", [WPC, NT, CH], fp32, kind="ExternalOutput")

    with tile.TileContext(nc) as tc:
        with (
            tc.tile_pool(name="const", bufs=1) as cpool,
            tc.tile_pool(name="work", bufs=2) as wpool,
            tc.tile_pool(name="attn", bufs=4) as apool,
            tc.tile_pool(name="ps_qk", bufs=2, space="PSUM") as ps_qk,
            tc.tile_pool(name="ps_av", bufs=2, space="PSUM") as ps_av,
            tc.tile_pool(name="ps_ms", bufs=2, space="PSUM") as ps_ms,
        ):
            # ---------------- one-time setup ----------------
            # persistent fp8 q,k feature tiles (ping-pong across pairs).
            # Slot a=1 stays all-zero: it is the second k-tile of every
            # DoubleRow QK^T matmul (the PE streams 2 tiles/cycle in DR mode,
            # so a 32-deep contraction runs at 0.5 cyc/row with a zero mate).
            # First tile zeroed on idle DVE right away (needed ~4us in);
            # second on Pool (not needed until pair 1).
            qk8s = []
            for pp_ in range(2):
                qk8t = cpool.tile([128, 2, 8, 2 * NT], f8, tag=f"qk8_{pp_}",
                                  name=f"qk8_{pp_}")
                if pp_ == 0:
                    nc.vector.memset(qk8t[:, 1, :, :], 0.0)
                else:
                    nc.gpsimd.memset(qk8t[:, 1, :, :], 0.0)
                qk8s.append(qk8t)

            # x loads own the SP queue; weights go via ACT hwdge (in
            # first-needed order); small/late tensors via gpsimd swdge.
            def load_x(pair):
                x8 = wpool.tile([128, 4, 2 * NT], f8, tag=f"x8{pair % 2}")
                nc.sync.dma_start(x8[:], x8d.ap()[pair])
                xT = wpool.tile([128, 4, 2 * NT], bf16, tag=f"xT{pair % 2}")
                nc.sync.dma_start(xT[:], x.ap()[pair])
                return xT, x8

            xs_cur = load_x(0)

            wqk = cpool.tile([128, 4, 2 * CH], f8, tag="wqk")
            nc.scalar.dma_start(wqk[:], wqk_d.ap().rearrange("(ko ki) m -> ki ko m", ki=128))
            wv = cpool.tile([128, 4, CH], bf16, tag="wv")
            nc.scalar.dma_start(wv[:], wv_d.ap().rearrange("(ko ki) m -> ki ko m", ki=128))
            # ebt in 4-head chunks so the big table transfer never delays the
            # startup-critical x/weight DMAs; chunk g unblocks groups in order
            ebt = cpool.tile([128, H, 2 * NT], bf16, tag="ebt")
            for g4 in range(4):
                nc.gpsimd.dma_start(ebt[:, 4 * g4:4 * g4 + 4, :],
                                    ebt_d.ap()[:, 4 * g4:4 * g4 + 4, :])
            pw = cpool.tile([128, 4, CH], bf16, tag="pw")
            nc.scalar.dma_start(pw[:], proj_w.ap().rearrange("(ko ki) m -> ki ko m", ki=128))

            b_row = cpool.tile([1, CH], fp32, tag="brow")
            nc.gpsimd.dma_start(b_row[:], proj_b.ap())
            b_bcast = cpool.tile([128, CH], fp32, tag="bb")
            nc.gpsimd.partition_broadcast(b_bcast[:], b_row[:], channels=128)

            ones32 = cpool.tile([128, 32], bf16, tag="ones")
            nc.gpsimd.memset(ones32[:], 1.0)

            pending = []  # deferred proj emission (fills next pair's qkv stage)

            def emit_qk(qk8, wi, sg):
                wo = wi * NT
                qkps = ps_qk.tile([128, 2, 512], fp32, tag="qkps")
                for j in range(2):
                    h = 2 * sg + j
                    hb = 32 * (h % 4)
                    qblk, kblk = h // 4, 4 + h // 4
                    rhs_q = qk8[hb:hb + 32, :, qblk, wo:wo + NT]
                    nc.tensor.matmul(qkps[:, j, 0:NT],
                                     qk8[hb:hb + 32, :, kblk, wo:wo + 128],
                                     rhs_q, start=True, stop=True,
                                     tile_position=(hb, 0), perf_mode=DR)
                    if wi == 0:
                        nc.tensor.matmul(qkps[:, j, NT:2 * NT],
                                         qk8[hb:hb + 32, :, kblk, 128:256],
                                         rhs_q, start=True, stop=True,
                                         tile_position=(hb, 0), perf_mode=DR)
                    else:
                        nc.tensor.matmul(qkps[0:68, j, NT:2 * NT],
                                         qk8[hb:hb + 32, :, kblk, wo + 128:wo + NT],
                                         rhs_q, start=True, stop=True,
                                         tile_position=(hb, 0), perf_mode=DR)
                return qkps

            def attn_window(qk8, v_sb, wi, qlist):
                # qlist: already-emitted qkps tiles (pipeline pre-fill)
                P = len(qlist)
                # one tile per 4-head group so proj block bl only depends on
                # its own group's normalize (not the last one)
                attn_rs = [apool.tile([128, NT], bf16, tag=f"attn_r{g}",
                                      name=f"attn_r{g}")
                           for g in range(4)]
                avps = None
                for sg in range(8):
                    qkps = qlist[sg]
                    # exp (ACT) then *exp(bias) (DVE, bf16 2x); the softmax
                    # 1/sqrt(d) rides along as the activation pre-scale
                    esb = apool.tile([128, 2, 2 * NT], bf16, tag="esb")
                    nc.scalar.activation(esb[:], qkps[:, :, 0:2 * NT], AF.Exp,
                                         scale=0.17677669529663687)
                    et = apool.tile([128, 2, 2 * NT], bf16, tag="et")
                    nc.vector.tensor_mul(et[:], esb[:], ebt[:, 2 * sg:2 * sg + 2, :])
                    if sg + P < 8:
                        qlist.append(emit_qk(qk8, wi, sg + P))
                    # AV + replicated denominators into 4-head psum bank
                    if sg % 2 == 0:
                        avps = ps_av.tile([128, 512], fp32, tag="avps")
                    for j in range(2):
                        h = 2 * sg + j
                        band = 32 * (h % 4)
                        nc.tensor.matmul(avps[band:band + 32, 0:NT],
                                         v_sb[:, 0, h, :], et[:, j, 0:NT],
                                         start=True, stop=False,
                                         tile_position=(0, band))
                        nc.tensor.matmul(avps[band:band + 32, 0:NT],
                                         v_sb[0:68, 1, h, :], et[0:68, j, NT:2 * NT],
                                         start=False, stop=True,
                                         tile_position=(0, band))
                        nc.tensor.matmul(avps[band:band + 32, 256:256 + NT],
                                         ones32[:], et[:, j, 0:NT],
                                         start=True, stop=False,
                                         tile_position=(0, band))
                        nc.tensor.matmul(avps[band:band + 32, 256:256 + NT],
                                         ones32[0:68, :], et[0:68, j, NT:2 * NT],
                                         start=False, stop=True,
                                         tile_position=(0, band))
                    if sg % 2 == 1:
                        g = sg // 2
                        r_d = apool.tile([128, NT], fp16, tag="rd")
                        with nc.allow_low_precision(reason="softmax recip in fp16"):
                            nc.vector.reciprocal(r_d[:], avps[:, 256:256 + NT])
                        nc.vector.tensor_mul(attn_rs[g][:], avps[:, 0:NT], r_d[:])
                return attn_rs

            def proj_emit(w, wi, attn_rs):
                # deferred (w1) proj borrows the attention psum pool: during
                # the next pair's qkv stage it is idle, so its banks don't
                # steal the qpv rotation from under the matmuls
                for tch, tsz in ((0, 128), (1, 68)):
                    if wi == 0:
                        pp = ps_ms.tile([128, 512], fp32, tag="ms", name="pp")
                    else:
                        pp = ps_av.tile([128, 512], fp32, tag="avps", name="ppd")
                    for bl in range(4):
                        nc.tensor.matmul(pp[0:tsz, 0:CH],
                                         attn_rs[bl][:, tch * 128:tch * 128 + tsz],
                                         pw[:, bl, :], start=(bl == 0), stop=(bl == 3))
                    yt = wpool.tile([128, CH], fp32, tag=f"yt{wi}{tch}")
                    nc.vector.tensor_add(yt[0:tsz, :], pp[0:tsz, 0:CH], b_bcast[0:tsz, :])
                    nc.sync.dma_start(y.ap()[w, tch * 128:tch * 128 + tsz, :], yt[0:tsz, :])

            # ---------------- main loop ----------------
            for pair in range(WPC // 2):
                # prefetch next pair's x before anything else hits SP's queue
                xs_next = load_x(pair + 1) if pair + 1 < WPC // 2 else None
                xT, x8 = xs_cur

                # q,k feature-major fp8 [128, slot, blk, 392] (fp8 DoubleRow
                # matmuls); softmax 1/sqrt(d) is applied later in the exp.
                qk8 = qk8s[pair % 2]
                q0 = []
                for mb in range(8):
                    qpv = ps_ms.tile([128, 512], fp32, tag="ms", name="qpv")
                    for t in range(2):
                        nc.tensor.matmul(qpv[:, 0:2 * NT],
                                         wqk[:, 2 * t:2 * t + 2, mb * 128:(mb + 1) * 128],
                                         x8[:, 2 * t:2 * t + 2, :],
                                         start=(t == 0), stop=(t == 1), perf_mode=DR)
                    # undo the host fp8 weight pre-scale (x64). Evacs alternate
                    # ACT/DVE: the DR matmuls are so cheap the stage is
                    # evac-paced, so one engine alone would bottleneck it.
                    if mb % 2 == 0:
                        nc.scalar.activation(qk8[:, 0, mb, :], qpv[:, 0:2 * NT],
                                             AF.Copy, scale=1.0 / 64.0)
                    else:
                        nc.vector.tensor_scalar_mul(qk8[:, 0, mb, :], qpv[:, 0:2 * NT],
                                                    1.0 / 64.0)
                    if mb == 2 and pending:
                        pending.pop()()  # prev pair's w1 proj fills this stage
                    if mb == 4:
                        # w0's first two QK sub-groups only need feature
                        # blocks 0 and 4 — start them mid-stage so the exp
                        # chain drains while the rest of qkv runs
                        q0.append(emit_qk(qk8, 0, 0))
                    elif mb == 5:
                        q0.append(emit_qk(qk8, 0, 1))

                # v token-major [128(tok), 2(chunk), H, D] bf16, per window
                vs = [None, None]
                for wi in range(2):
                    wo = wi * NT
                    v_sb = wpool.tile([128, 2, H, D], bf16, tag=f"v{wi}")
                    vs[wi] = v_sb
                    for tch, tsz in ((0, 128), (1, 68)):
                        vpv = ps_ms.tile([128, 512], fp32, tag="ms", name="vpv")
                        for kc in range(4):
                            nc.tensor.matmul(
                                vpv[0:tsz, 0:CH],
                                xT[:, kc, wo + tch * 128: wo + tch * 128 + tsz],
                                wv[:, kc, :],
                                start=(kc == 0), stop=(kc == 3))
                        # split the two chunk evacs across ACT/Pool so v_sb is
                        # ready before the first AV matmul needs it and DVE
                        # stays free for the softmax stream
                        if tch == 0:
                            nc.scalar.activation(
                                v_sb[0:tsz, tch, :, :].rearrange("p h d -> p (h d)"),
                                vpv[0:tsz, 0:CH], AF.Copy)
                        else:
                            nc.vector.tensor_copy(
                                v_sb[0:tsz, tch, :, :].rearrange("p h d -> p (h d)"),
                                vpv[0:tsz, 0:CH])

                attn_r0 = attn_window(qk8, vs[0], 0, q0)
                q1 = [emit_qk(qk8, 1, 0)]
                proj_emit(2 * pair, 0, attn_r0)
                attn_r1 = attn_window(qk8, vs[1], 1, q1)
                if pair == WPC // 2 - 1:
                    proj_emit(2 * pair + 1, 1, attn_r1)
                else:
                    pending.append(
                        lambda w=2 * pair + 1, a=attn_r1: proj_emit(w, 1, a))

                xs_cur = xs_next

            while pending:
                pending.pop()()

    nc.compile()
    return nc


def _prep_ebt(rel_pos_index, rel_bias_table):
    # ebt[p, h, khi*196 + q] = exp(table[idx[q, p + 128*khi], h]) (1.0 where k pad)
    idx = np.asarray(rel_pos_index).astype(np.int64)
    table = np.asarray(rel_bias_table, dtype=np.float32)
    g = table[idx]                      # [q, k, H]
    out = np.zeros((256, H, NT), dtype=np.float32)
    out[:NT] = g.transpose(1, 2, 0)     # [k, H, q]
    out = np.exp(out)
    return np.ascontiguousarray(
        out.reshape(2, 128, H, NT).transpose(1, 2, 0, 3).reshape(128, H, 2 * NT))


def kernel(x, qkv_w, rel_bias_table, proj_w, proj_b, rel_pos_index):
    import ml_dtypes
    from concourse.bass_utils import run_bass_kernel_spmd

    if "nc" not in _CACHE:
        _CACHE["nc"] = _build()
    nc = _CACHE["nc"]

    bf16 = ml_dtypes.bfloat16
    f8 = ml_dtypes.float8_e4m3
    # host pre-transpose: [b/2, ki, ko, wi*196+q] = x[w, q, 128*ko+ki]
    xf = np.asarray(x, dtype=np.float32).reshape(B // 2, 2, NT, 4, 128)
    xf = np.ascontiguousarray(xf.transpose(0, 4, 3, 1, 2).reshape(B // 2, 128, 4, 2 * NT))
    x16 = xf.astype(bf16)
    x8 = xf.astype(f8)
    qkv_f = np.asarray(qkv_w, dtype=np.float32)
    # x64 pre-scale lifts the tiny weights out of fp8 subnormal range;
    # undone (with the softmax scale for q) in the on-device evacuation.
    wqk = np.ascontiguousarray(qkv_f[:, :2 * CH] * 64.0).astype(f8)
    wv = np.ascontiguousarray(qkv_f[:, 2 * CH:]).astype(bf16)
    ebt = _prep_ebt(rel_pos_index, rel_bias_table).astype(bf16)
    pw = np.ascontiguousarray(np.asarray(proj_w, dtype=np.float32)).astype(bf16)
    pb = np.ascontiguousarray(np.asarray(proj_b), dtype=np.float32).reshape(1, CH)

    hw = WPC // 2
    in_maps = []
    for c in range(NCORES):
        in_maps.append({
            "x": x16[c * hw:(c + 1) * hw],
            "x8": x8[c * hw:(c + 1) * hw],
            "wqk": wqk,
            "wv": wv,
            "ebt": ebt,
            "proj_w": pw,
            "proj_b": pb,
        })
    res = run_bass_kernel_spmd(nc, in_maps, core_ids=list(range(NCORES)))
    out = np.concatenate([r["y"] for r in res.results], axis=0)
    return out.astype(np.float32)


if __name__ == "__main__":
    pass


# revision 5
# speedup vs baseline: 1.0073x; 1.0069x over previous
"""Swin-style window attention kernel for 8 TRN2 NeuronCores (SPMD, batch-sharded).

Layout strategy per core (16 windows, processed in pairs):
  - host prep: x pre-transposed to [pair, ki, ko, tok] in both fp8e4m3 and
    bf16; q,k weights x64-scaled to fp8 (rescaled in the on-device evac);
    v/proj weights bf16; exp(rel-pos bias) precomputed as a bf16 table.
  - q,k projection: fp8 DoubleRow matmuls (2 k-tiles/instr, 0.5 cyc/row),
    evacuated fp8 into persistent ping-pong tiles whose second slot is
    all-zero - the zero slot is the mate k-tile that lets the 32-deep
    QK^T contraction also run in DoubleRow mode at 0.5 cyc/row.
  - v projection bf16, token-major [tok,(h,d)].
  - attention in 2-head sub-groups, double-buffered QK psum: QK^T
    transposed (attnT [k,196]) fp8-DR 32-row-packed matmuls; exp on ACT
    with the softmax 1/sqrt(d) as activation pre-scale; *exp(bias) on DVE
    (bf16 2x); AV dense + replicated denominators via ones-matmuls into a
    4-head psum bank; reciprocal + normalize on DVE into per-group
    attn_r tiles (so proj block g depends only on group g).
  - pipeline: w0's first QK sub-groups issued mid-qkv; QK(sg+2) staggered
    ahead of AV(sg); w0 proj inline after w1's first QK; w1 proj deferred
    into the next pair's qkv stage on the idle attention-psum pool.
  - proj bf16, bias added during psum evacuation (DVE); y stores + x
    prefetch on the SP queue, weights on ACT hwdge, exp(bias) table in
    4-head chunks on gpsimd swdge.
"""

import numpy as np

B, NT, CH = 128, 196, 512
H, D = 16, 32
NCORES = 8
WPC = B // NCORES  # windows per core

_CACHE = {}


def _build():
    import concourse.bass as bass
    import concourse.mybir as mybir
    import concourse.tile as tile
    from concourse import bacc

    fp32 = mybir.dt.float32
    bf16 = mybir.dt.bfloat16
    fp16 = mybir.dt.float16
    f8 = mybir.dt.float8e4
    DR = mybir.MatmulPerfMode.DoubleRow
    AF = mybir.ActivationFunctionType

    nc = bacc.Bacc("TRN2", target_bir_lowering=False, debug=False, num_devices=NCORES)

    # x pre-transposed on host: [pair, ki, ko, wi*196+q] = x[w, q, 128*ko+ki]
    # fp8 copy feeds the q,k projection (DoubleRow); bf16 copy feeds v.
    x8d = nc.dram_tensor("x8", [WPC // 2, 128, 4, 2 * NT], f8, kind="ExternalInput")
    x = nc.dram_tensor("x", [WPC // 2, 128, 4, 2 * NT], bf16, kind="ExternalInput")
    wqk_d = nc.dram_tensor("wqk", [CH, 2 * CH], f8, kind="ExternalInput")
    wv_d = nc.dram_tensor("wv", [CH, CH], bf16, kind="ExternalInput")
    ebt_d = nc.dram_tensor("ebt", [128, H, 2 * NT], bf16, kind="ExternalInput")
    proj_w = nc.dram_tensor("proj_w", [CH, CH], bf16, kind="ExternalInput")
    proj_b = nc.dram_tensor("proj_b", [1, CH], fp32, kind="ExternalInput")
    y = nc.dram_tensor("y", [WPC, NT, CH], fp32, kind="ExternalOutput")

    with tile.TileContext(nc) as tc:
        with (
            tc.tile_pool(name="const", bufs=1) as cpool,
            tc.tile_pool(name="work", bufs=2) as wpool,
            tc.tile_pool(name="attn", bufs=4) as apool,
            tc.tile_pool(name="ps_qk", bufs=2, space="PSUM") as ps_qk,
            tc.tile_pool(name="ps_av", bufs=2, space="PSUM") as ps_av,
            tc.tile_pool(name="ps_ms", bufs=2, space="PSUM") as ps_ms,
        ):
            # ---------------- one-time setup ----------------
            # persistent fp8 q,k feature tiles (ping-pong across pairs).
            # Slot a=1 stays all-zero: it is the second k-tile of every
            # DoubleRow QK^T matmul (the PE streams 2 tiles/cycle in DR mode,
            # so a 32-deep contraction runs at 0.5 cyc/row with a zero mate).
            # First tile zeroed on idle DVE right away (needed ~4us in);
            # second on Pool (not needed until pair 1).
            qk8s = []
            for pp_ in range(2):
                qk8t = cpool.tile([128, 2, 8, 2 * NT], f8, tag=f"qk8_{pp_}",
                                  name=f"qk8_{pp_}")
                if pp_ == 0:
                    nc.vector.memset(qk8t[:, 1, :, :], 0.0)
                else:
                    nc.gpsimd.memset(qk8t[:, 1, :, :], 0.0)
                qk8s.append(qk8t)

            # x loads own the SP queue; weights go via ACT hwdge (in
            # first-needed order); small/late tensors via gpsimd swdge.
            def load_x(pair):
                x8 = wpool.tile([128, 4, 2 * NT], f8, tag=f"x8{pair % 2}")
                nc.sync.dma_start(x8[:], x8d.ap()[pair])
                xT = wpool.tile([128, 4, 2 * NT], bf16, tag=f"xT{pair % 2}")
                nc.sync.dma_start(xT[:], x.ap()[pair])
                return xT, x8

            xs_cur = load_x(0)

            wqk = cpool.tile([128, 4, 2 * CH], f8, tag="wqk")
            nc.scalar.dma_start(wqk[:], wqk_d.ap().rearrange("(ko ki) m -> ki ko m", ki=128))
            wv = cpool.tile([128, 4, CH], bf16, tag="wv")
            nc.scalar.dma_start(wv[:], wv_d.ap().rearrange("(ko ki) m -> ki ko m", ki=128))
            # ebt in 4-head chunks so the big table transfer never delays the
            # startup-critical x/weight DMAs; chunk g unblocks groups in order
            ebt = cpool.tile([128, H, 2 * NT], bf16, tag="ebt")
            for g4 in range(4):
                nc.gpsimd.dma_start(ebt[:, 4 * g4:4 * g4 + 4, :],
                                    ebt_d.ap()[:, 4 * g4:4 * g4 + 4, :])
            pw = cpool.tile([128, 4, CH], bf16, tag="pw")
            nc.scalar.dma_start(pw[:], proj_w.ap().rearrange("(ko ki) m -> ki ko m", ki=128))

            b_row = cpool.tile([1, CH], fp32, tag="brow")
            nc.gpsimd.dma_start(b_row[:], proj_b.ap())
            b_bcast = cpool.tile([128, CH], fp32, tag="bb")
            nc.gpsimd.partition_broadcast(b_bcast[:], b_row[:], channels=128)

            ones32 = cpool.tile([128, 32], bf16, tag="ones")
            nc.gpsimd.memset(ones32[:], 1.0)

            pending = []  # deferred proj emission (fills next pair's qkv stage)

            def emit_qk(qk8, wi, sg):
                wo = wi * NT
                qkps = ps_qk.tile([128, 2, 512], fp32, tag="qkps")
                for j in range(2):
                    h = 2 * sg + j
                    hb = 32 * (h % 4)
                    qblk, kblk = h // 4, 4 + h // 4
                    rhs_q = qk8[hb:hb + 32, :, qblk, wo:wo + NT]
                    nc.tensor.matmul(qkps[:, j, 0:NT],
                                     qk8[hb:hb + 32, :, kblk, wo:wo + 128],
                                     rhs_q, start=True, stop=True,
                                     tile_position=(hb, 0), perf_mode=DR)
                    if wi == 0:
                        nc.tensor.matmul(qkps[:, j, NT:2 * NT],
                                         qk8[hb:hb + 32, :, kblk, 128:256],
                                         rhs_q, start=True, stop=True,
                                         tile_position=(hb, 0), perf_mode=DR)
                    else:
                        nc.tensor.matmul(qkps[0:68, j, NT:2 * NT],
                                         qk8[hb:hb + 32, :, kblk, wo + 128:wo + NT],
                                         rhs_q, start=True, stop=True,
                                         tile_position=(hb, 0), perf_mode=DR)
                return qkps

            def attn_window(qk8, v_sb, wi, qlist):
                # qlist: already-emitted qkps tiles (pipeline pre-fill)
                P = len(qlist)
                # one tile per 4-head group so proj block bl only depends on
                # its own group's normalize (not the last one)
                attn_rs = [apool.tile([128, NT], bf16, tag=f"attn_r{g}",
                                      name=f"attn_r{g}")
                           for g in range(4)]
                avps = None
                for sg in range(8):
                    qkps = qlist[sg]
                    # exp (ACT) then *exp(bias) (DVE, bf16 2x); the softmax
                    # 1/sqrt(d) rides along as the activation pre-scale
                    esb = apool.tile([128, 2, 2 * NT], bf16, tag="esb")
                    nc.scalar.activation(esb[:], qkps[:, :, 0:2 * NT], AF.Exp,
                                         scale=0.17677669529663687)
                    et = apool.tile([128, 2, 2 * NT], bf16, tag="et")
                    nc.vector.tensor_mul(et[:], esb[:], ebt[:, 2 * sg:2 * sg + 2, :])
                    if sg + P < 8:
                        qlist.append(emit_qk(qk8, wi, sg + P))
                    # AV + replicated denominators into 4-head psum bank
                    if sg % 2 == 0:
                        avps = ps_av.tile([128, 512], fp32, tag="avps")
                    # ones (denominator) matmuls FIRST: the reciprocal can
                    # then overlap the AV matmuls instead of serializing
                    # after them on the window-end critical path
                    for j in range(2):
                        h = 2 * sg + j
                        band = 32 * (h % 4)
                        nc.tensor.matmul(avps[band:band + 32, 256:256 + NT],
                                         ones32[:], et[:, j, 0:NT],
                                         start=True, stop=False,
                                         tile_position=(0, band))
                        nc.tensor.matmul(avps[band:band + 32, 256:256 + NT],
                                         ones32[0:68, :], et[0:68, j, NT:2 * NT],
                                         start=False, stop=True,
                                         tile_position=(0, band))
                    for j in range(2):
                        h = 2 * sg + j
                        band = 32 * (h % 4)
                        nc.tensor.matmul(avps[band:band + 32, 0:NT],
                                         v_sb[:, 0, h, :], et[:, j, 0:NT],
                                         start=True, stop=False,
                                         tile_position=(0, band))
                        nc.tensor.matmul(avps[band:band + 32, 0:NT],
                                         v_sb[0:68, 1, h, :], et[0:68, j, NT:2 * NT],
                                         start=False, stop=True,
                                         tile_position=(0, band))
                    if sg % 2 == 1:
                        g = sg // 2
                        r_d = apool.tile([128, NT], fp16, tag="rd")
                        with nc.allow_low_precision(reason="softmax recip in fp16"):
                            nc.vector.reciprocal(r_d[:], avps[:, 256:256 + NT])
                        nc.vector.tensor_mul(attn_rs[g][:], avps[:, 0:NT], r_d[:])
                return attn_rs

            def proj_emit(w, wi, attn_rs):
                # deferred (w1) proj borrows the attention psum pool: during
                # the next pair's qkv stage it is idle, so its banks don't
                # steal the qpv rotation from under the matmuls
                for tch, tsz in ((0, 128), (1, 68)):
                    if wi == 0:
                        pp = ps_ms.tile([128, 512], fp32, tag="ms", name="pp")
                    else:
                        pp = ps_av.tile([128, 512], fp32, tag="avps", name="ppd")
                    for bl in range(4):
                        nc.tensor.matmul(pp[0:tsz, 0:CH],
                                         attn_rs[bl][:, tch * 128:tch * 128 + tsz],
                                         pw[:, bl, :], start=(bl == 0), stop=(bl == 3))
                    yt = wpool.tile([128, CH], fp32, tag=f"yt{wi}{tch}")
                    nc.vector.tensor_add(yt[0:tsz, :], pp[0:tsz, 0:CH], b_bcast[0:tsz, :])
                    nc.sync.dma_start(y.ap()[w, tch * 128:tch * 128 + tsz, :], yt[0:tsz, :])

            # ---------------- main loop ----------------
            for pair in range(WPC // 2):
                # prefetch next pair's x before anything else hits SP's queue
                xs_next = load_x(pair + 1) if pair + 1 < WPC // 2 else None
                xT, x8 = xs_cur

                # q,k feature-major fp8 [128, slot, blk, 392] (fp8 DoubleRow
                # matmuls); softmax 1/sqrt(d) is applied later in the exp.
                qk8 = qk8s[pair % 2]
                q0 = []
                for mb in range(8):
                    qpv = ps_ms.tile([128, 512], fp32, tag="ms", name="qpv")
                    for t in range(2):
                        nc.tensor.matmul(qpv[:, 0:2 * NT],
                                         wqk[:, 2 * t:2 * t + 2, mb * 128:(mb + 1) * 128],
                                         x8[:, 2 * t:2 * t + 2, :],
                                         start=(t == 0), stop=(t == 1), perf_mode=DR)
                    # undo the host fp8 weight pre-scale (x64). Evacs alternate
                    # ACT/DVE: the DR matmuls are so cheap the stage is
                    # evac-paced, so one engine alone would bottleneck it.
                    if mb % 2 == 0:
                        nc.scalar.activation(qk8[:, 0, mb, :], qpv[:, 0:2 * NT],
                                             AF.Copy, scale=1.0 / 64.0)
                    else:
                        nc.vector.tensor_scalar_mul(qk8[:, 0, mb, :], qpv[:, 0:2 * NT],
                                                    1.0 / 64.0)
                    if mb == 2 and pending:
                        pending.pop()()  # prev pair's w1 proj fills this stage
                    if mb == 4:
                        # w0's first two QK sub-groups only need feature
                        # blocks 0 and 4 — start them mid-stage so the exp
                        # chain drains while the rest of qkv runs
                        q0.append(emit_qk(qk8, 0, 0))
                    elif mb == 5:
                        q0.append(emit_qk(qk8, 0, 1))

                # v token-major [128(tok), 2(chunk), H, D] bf16, per window
                vs = [None, None]
                for wi in range(2):
                    wo = wi * NT
                    v_sb = wpool.tile([128, 2, H, D], bf16, tag=f"v{wi}")
                    vs[wi] = v_sb
                    for tch, tsz in ((0, 128), (1, 68)):
                        vpv = ps_ms.tile([128, 512], fp32, tag="ms", name="vpv")
                        for kc in range(4):
                            nc.tensor.matmul(
                                vpv[0:tsz, 0:CH],
                                xT[:, kc, wo + tch * 128: wo + tch * 128 + tsz],
                                wv[:, kc, :],
                                start=(kc == 0), stop=(kc == 3))
                        # split the two chunk evacs across ACT/Pool so v_sb is
                        # ready before the first AV matmul needs it and DVE
                        # stays free for the softmax stream
                        if tch == 0:
                            nc.scalar.activation(
                                v_sb[0:tsz, tch, :, :].rearrange("p h d -> p (h d)"),
                                vpv[0:tsz, 0:CH], AF.Copy)
                        else:
                            nc.vector.tensor_copy(
                                v_sb[0:tsz, tch, :, :].rearrange("p h d -> p (h d)"),
                                vpv[0:tsz, 0:CH])

                attn_r0 = attn_window(qk8, vs[0], 0, q0)
                q1 = [emit_qk(qk8, 1, 0)]
                proj_emit(2 * pair, 0, attn_r0)
                attn_r1 = attn_window(qk8, vs[1], 1, q1)
                if pair == WPC // 2 - 1:
                    proj_emit(2 * pair + 1, 1, attn_r1)
                else:
                    pending.append(
                        lambda w=2 * pair + 1, a=attn_r1: proj_emit(w, 1, a))

                xs_cur = xs_next

            while pending:
                pending.pop()()

    nc.compile()
    return nc


def _prep_ebt(rel_pos_index, rel_bias_table):
    # ebt[p, h, khi*196 + q] = exp(table[idx[q, p + 128*khi], h]) (1.0 where k pad)
    idx = np.asarray(rel_pos_index).astype(np.int64)
    table = np.asarray(rel_bias_table, dtype=np.float32)
    g = table[idx]                      # [q, k, H]
    out = np.zeros((256, H, NT), dtype=np.float32)
    out[:NT] = g.transpose(1, 2, 0)     # [k, H, q]
    out = np.exp(out)
    return np.ascontiguousarray(
        out.reshape(2, 128, H, NT).transpose(1, 2, 0, 3).reshape(128, H, 2 * NT))


def kernel(x, qkv_w, rel_bias_table, proj_w, proj_b, rel_pos_index):
    import ml_dtypes
    from concourse.bass_utils import run_bass_kernel_spmd

    if "nc" not in _CACHE:
        _CACHE["nc"] = _build()
    nc = _CACHE["nc"]

    bf16 = ml_dtypes.bfloat16
    f8 = ml_dtypes.float8_e4m3
    # host pre-transpose: [b/2, ki, ko, wi*196+q] = x[w, q, 128*ko+ki]
    xf = np.asarray(x, dtype=np.float32).reshape(B // 2, 2, NT, 4, 128)
    xf = np.ascontiguousarray(xf.transpose(0, 4, 3, 1, 2).reshape(B // 2, 128, 4, 2 * NT))
    x16 = xf.astype(bf16)
    x8 = xf.astype(f8)
    qkv_f = np.asarray(qkv_w, dtype=np.float32)
    # x64 pre-scale lifts the tiny weights out of fp8 subnormal range;
    # undone (with the softmax scale for q) in the on-device evacuation.
    wqk = np.ascontiguousarray(qkv_f[:, :2 * CH] * 64.0).astype(f8)
    wv = np.ascontiguousarray(qkv_f[:, 2 * CH:]).astype(bf16)
    ebt = _prep_ebt(rel_pos_index, rel_bias_table).astype(bf16)
    pw = np.ascontiguousarray(np.asarray(proj_w, dtype=np.float32)).astype(bf16)
    pb = np.ascontiguousarray(np.asarray(proj_b), dtype=np.float32).reshape(1, CH)

    hw = WPC // 2
    in_maps = []
    for c in range(NCORES):
        in_maps.append({
            "x": x16[c * hw:(c + 1) * hw],
            "x8": x8[c * hw:(c + 1) * hw],
            "wqk": wqk,
            "wv": wv,
            "ebt": ebt,
            "proj_w": pw,
            "proj_b": pb,
        })
    res = run_bass_kernel_spmd(nc, in_maps, core_ids=list(range(NCORES)))
    out = np.concatenate([r["y"] for r in res.results], axis=0)
    return out.astype(np.float32)


if __name__ == "__main__":
    pass


# revision 7
# speedup vs baseline: 1.0221x; 1.0147x over previous
"""Swin-style window attention kernel for 8 TRN2 NeuronCores (SPMD, batch-sharded).

Layout strategy per core (16 windows, processed in pairs):
  - host prep: x pre-transposed to [pair, ki, ko, tok] in both fp8e4m3 and
    bf16; q,k weights x64-scaled to fp8 (rescaled in the on-device evac);
    v/proj weights bf16; exp(rel-pos bias) precomputed as a bf16 table.
  - q,k projection: fp8 DoubleRow matmuls (2 k-tiles/instr, 0.5 cyc/row),
    evacuated fp8 into persistent ping-pong tiles whose second slot is
    all-zero - the zero slot is the mate k-tile that lets the 32-deep
    QK^T contraction also run in DoubleRow mode at 0.5 cyc/row.
  - v projection bf16, token-major [tok,(h,d)].
  - attention in 2-head sub-groups, double-buffered QK psum: QK^T
    transposed (attnT [k,196]) fp8-DR 32-row-packed matmuls; exp on ACT
    with the softmax 1/sqrt(d) as activation pre-scale; *exp(bias) on DVE
    (bf16 2x); AV dense + replicated denominators via ones-matmuls into a
    4-head psum bank; reciprocal + normalize on DVE into per-group
    attn_r tiles (so proj block g depends only on group g).
  - pipeline: w0's first QK sub-groups issued mid-qkv; QK(sg+2) staggered
    ahead of AV(sg); w0 proj inline after w1's first QK; w1 proj deferred
    into the next pair's qkv stage on the idle attention-psum pool.
  - proj bf16, bias added during psum evacuation (DVE); y stores + x
    prefetch on the SP queue, weights on ACT hwdge, exp(bias) table in
    4-head chunks on gpsimd swdge.
"""

import numpy as np

B, NT, CH = 128, 196, 512
H, D = 16, 32
NCORES = 8
WPC = B // NCORES  # windows per core

_CACHE = {}


def _build():
    import concourse.bass as bass
    import concourse.mybir as mybir
    import concourse.tile as tile
    from concourse import bacc

    fp32 = mybir.dt.float32
    bf16 = mybir.dt.bfloat16
    fp16 = mybir.dt.float16
    f8 = mybir.dt.float8e4
    DR = mybir.MatmulPerfMode.DoubleRow
    AF = mybir.ActivationFunctionType

    nc = bacc.Bacc("TRN2", target_bir_lowering=False, debug=False, num_devices=NCORES)

    # x pre-transposed on host: [pair, ki, ko, wi*196+q] = x[w, q, 128*ko+ki]
    # fp8 copy feeds the q,k projection (DoubleRow); bf16 copy feeds v.
    x8d = nc.dram_tensor("x8", [WPC // 2, 128, 4, 2 * NT], f8, kind="ExternalInput")
    x = nc.dram_tensor("x", [WPC // 2, 128, 4, 2 * NT], bf16, kind="ExternalInput")
    wqk_d = nc.dram_tensor("wqk", [CH, 2 * CH], f8, kind="ExternalInput")
    wv_d = nc.dram_tensor("wv", [CH, CH], bf16, kind="ExternalInput")
    ebt_d = nc.dram_tensor("ebt", [128, H, 2 * NT], bf16, kind="ExternalInput")
    proj_w = nc.dram_tensor("proj_w", [CH, CH], bf16, kind="ExternalInput")
    proj_b = nc.dram_tensor("proj_b", [1, CH], fp32, kind="ExternalInput")
    y = nc.dram_tensor("y", [WPC, NT, CH], fp32, kind="ExternalOutput")

    with tile.TileContext(nc) as tc:
        with (
            tc.tile_pool(name="const", bufs=1) as cpool,
            tc.tile_pool(name="work", bufs=2) as wpool,
            tc.tile_pool(name="attn", bufs=4) as apool,
            tc.tile_pool(name="ps_qk", bufs=2, space="PSUM") as ps_qk,
            tc.tile_pool(name="ps_av", bufs=2, space="PSUM") as ps_av,
            tc.tile_pool(name="ps_ms", bufs=2, space="PSUM") as ps_ms,
        ):
            # ---------------- one-time setup ----------------
            # persistent fp8 q,k feature tiles (ping-pong across pairs).
            # Slot a=1 stays all-zero: it is the second k-tile of every
            # DoubleRow QK^T matmul (the PE streams 2 tiles/cycle in DR mode,
            # so a 32-deep contraction runs at 0.5 cyc/row with a zero mate).
            # First tile zeroed on idle DVE right away (needed ~4us in);
            # second on Pool (not needed until pair 1).
            qk8s = []
            for pp_ in range(2):
                qk8t = cpool.tile([128, 2, 8, 2 * NT], f8, tag=f"qk8_{pp_}",
                                  name=f"qk8_{pp_}")
                if pp_ == 0:
                    nc.vector.memset(qk8t[:, 1, :, :], 0.0)
                else:
                    nc.gpsimd.memset(qk8t[:, 1, :, :], 0.0)
                qk8s.append(qk8t)

            # x loads own the SP queue; weights go via ACT hwdge (in
            # first-needed order); small/late tensors via gpsimd swdge.
            def load_x(pair):
                # x8 split at the DoubleRow kt boundary: the first qkv
                # matmul only needs chunks 0-1, so it can start ~1us sooner
                x8 = wpool.tile([128, 4, 2 * NT], f8, tag=f"x8{pair % 2}")
                nc.sync.dma_start(x8[:, 0:2, :], x8d.ap()[pair, :, 0:2, :])
                nc.sync.dma_start(x8[:, 2:4, :], x8d.ap()[pair, :, 2:4, :])
                xT = wpool.tile([128, 4, 2 * NT], bf16, tag=f"xT{pair % 2}")
                nc.sync.dma_start(xT[:], x.ap()[pair])
                return xT, x8

            xs_cur = load_x(0)

            wqk = cpool.tile([128, 4, 2 * CH], f8, tag="wqk")
            wqk_r = wqk_d.ap().rearrange("(ko ki) m -> ki ko m", ki=128)
            nc.scalar.dma_start(wqk[:, 0:2, :], wqk_r[:, 0:2, :])
            nc.scalar.dma_start(wqk[:, 2:4, :], wqk_r[:, 2:4, :])
            wv = cpool.tile([128, 4, CH], bf16, tag="wv")
            nc.scalar.dma_start(wv[:], wv_d.ap().rearrange("(ko ki) m -> ki ko m", ki=128))
            # ebt in 4-head chunks so the big table transfer never delays the
            # startup-critical x/weight DMAs; chunk g unblocks groups in order
            ebt = cpool.tile([128, H, 2 * NT], bf16, tag="ebt")
            for g4 in range(4):
                nc.gpsimd.dma_start(ebt[:, 4 * g4:4 * g4 + 4, :],
                                    ebt_d.ap()[:, 4 * g4:4 * g4 + 4, :])
            pw = cpool.tile([128, 4, CH], bf16, tag="pw")
            nc.scalar.dma_start(pw[:], proj_w.ap().rearrange("(ko ki) m -> ki ko m", ki=128))

            b_row = cpool.tile([1, CH], fp32, tag="brow")
            nc.gpsimd.dma_start(b_row[:], proj_b.ap())
            b_bcast = cpool.tile([128, CH], fp32, tag="bb")
            nc.gpsimd.partition_broadcast(b_bcast[:], b_row[:], channels=128)

            ones32 = cpool.tile([128, 32], bf16, tag="ones")
            nc.gpsimd.memset(ones32[:], 1.0)

            pending = []  # deferred proj emission (fills next pair's qkv stage)

            def emit_qk(qk8, wi, sg):
                wo = wi * NT
                qkps = ps_qk.tile([128, 2, 512], fp32, tag="qkps")
                for j in range(2):
                    h = 2 * sg + j
                    hb = 32 * (h % 4)
                    qblk, kblk = h // 4, 4 + h // 4
                    rhs_q = qk8[hb:hb + 32, :, qblk, wo:wo + NT]
                    nc.tensor.matmul(qkps[:, j, 0:NT],
                                     qk8[hb:hb + 32, :, kblk, wo:wo + 128],
                                     rhs_q, start=True, stop=True,
                                     tile_position=(hb, 0), perf_mode=DR)
                    if wi == 0:
                        nc.tensor.matmul(qkps[:, j, NT:2 * NT],
                                         qk8[hb:hb + 32, :, kblk, 128:256],
                                         rhs_q, start=True, stop=True,
                                         tile_position=(hb, 0), perf_mode=DR)
                    else:
                        nc.tensor.matmul(qkps[0:68, j, NT:2 * NT],
                                         qk8[hb:hb + 32, :, kblk, wo + 128:wo + NT],
                                         rhs_q, start=True, stop=True,
                                         tile_position=(hb, 0), perf_mode=DR)
                return qkps

            def attn_window(qk8, v_sb, wi, qlist):
                # qlist: already-emitted qkps tiles (pipeline pre-fill)
                P = len(qlist)
                # one tile per 4-head group so proj block bl only depends on
                # its own group's normalize (not the last one)
                attn_rs = [apool.tile([128, NT], bf16, tag=f"attn_r{g}",
                                      name=f"attn_r{g}")
                           for g in range(4)]
                avps = None
                for sg in range(8):
                    qkps = qlist[sg]
                    # exp (ACT) then *exp(bias) (DVE, bf16 2x); the softmax
                    # 1/sqrt(d) rides along as the activation pre-scale
                    esb = apool.tile([128, 2, 2 * NT], bf16, tag="esb")
                    nc.scalar.activation(esb[:], qkps[:, :, 0:2 * NT], AF.Exp,
                                         scale=0.17677669529663687)
                    et = apool.tile([128, 2, 2 * NT], bf16, tag="et")
                    nc.vector.tensor_mul(et[:], esb[:], ebt[:, 2 * sg:2 * sg + 2, :])
                    if sg + P < 8:
                        qlist.append(emit_qk(qk8, wi, sg + P))
                    # AV + replicated denominators into 4-head psum bank
                    if sg % 2 == 0:
                        avps = ps_av.tile([128, 512], fp32, tag="avps")
                    # ones (denominator) matmuls FIRST: the reciprocal can
                    # then overlap the AV matmuls instead of serializing
                    # after them on the window-end critical path
                    for j in range(2):
                        h = 2 * sg + j
                        band = 32 * (h % 4)
                        nc.tensor.matmul(avps[band:band + 32, 256:256 + NT],
                                         ones32[:], et[:, j, 0:NT],
                                         start=True, stop=False,
                                         tile_position=(0, band))
                        nc.tensor.matmul(avps[band:band + 32, 256:256 + NT],
                                         ones32[0:68, :], et[0:68, j, NT:2 * NT],
                                         start=False, stop=True,
                                         tile_position=(0, band))
                    for j in range(2):
                        h = 2 * sg + j
                        band = 32 * (h % 4)
                        nc.tensor.matmul(avps[band:band + 32, 0:NT],
                                         v_sb[:, 0, h, :], et[:, j, 0:NT],
                                         start=True, stop=False,
                                         tile_position=(0, band))
                        nc.tensor.matmul(avps[band:band + 32, 0:NT],
                                         v_sb[0:68, 1, h, :], et[0:68, j, NT:2 * NT],
                                         start=False, stop=True,
                                         tile_position=(0, band))
                    if sg % 2 == 1:
                        g = sg // 2
                        r_d = apool.tile([128, NT], fp16, tag="rd")
                        with nc.allow_low_precision(reason="softmax recip in fp16"):
                            nc.vector.reciprocal(r_d[:], avps[:, 256:256 + NT])
                        nc.vector.tensor_mul(attn_rs[g][:], avps[:, 0:NT], r_d[:])
                return attn_rs

            def proj_emit(w, wi, attn_rs):
                # deferred (w1) proj borrows the attention psum pool: during
                # the next pair's qkv stage it is idle, so its banks don't
                # steal the qpv rotation from under the matmuls
                for tch, tsz in ((0, 128), (1, 68)):
                    if wi == 0:
                        pp = ps_ms.tile([128, 512], fp32, tag="ms", name="pp")
                    else:
                        pp = ps_av.tile([128, 512], fp32, tag="avps", name="ppd")
                    for bl in range(4):
                        nc.tensor.matmul(pp[0:tsz, 0:CH],
                                         attn_rs[bl][:, tch * 128:tch * 128 + tsz],
                                         pw[:, bl, :], start=(bl == 0), stop=(bl == 3))
                    yt = wpool.tile([128, CH], fp32, tag=f"yt{wi}{tch}")
                    nc.vector.tensor_add(yt[0:tsz, :], pp[0:tsz, 0:CH], b_bcast[0:tsz, :])
                    nc.sync.dma_start(y.ap()[w, tch * 128:tch * 128 + tsz, :], yt[0:tsz, :])

            # ---------------- main loop ----------------
            for pair in range(WPC // 2):
                # prefetch next pair's x before anything else hits SP's queue
                xs_next = load_x(pair + 1) if pair + 1 < WPC // 2 else None
                xT, x8 = xs_cur

                # q,k feature-major fp8 [128, slot, blk, 392] (fp8 DoubleRow
                # matmuls); softmax 1/sqrt(d) is applied later in the exp.
                qk8 = qk8s[pair % 2]
                q0 = []
                # block order (0,4,...) evacuates exactly the two blocks the
                # first QK sub-groups read (heads 0-3 use qblk 0, kblk 4), so
                # both prefills become legal after just two evacuations
                for mi, mb in enumerate((0, 4, 1, 5, 2, 6, 3, 7)):
                    qpv = ps_ms.tile([128, 512], fp32, tag="ms", name="qpv")
                    for t in range(2):
                        nc.tensor.matmul(qpv[:, 0:2 * NT],
                                         wqk[:, 2 * t:2 * t + 2, mb * 128:(mb + 1) * 128],
                                         x8[:, 2 * t:2 * t + 2, :],
                                         start=(t == 0), stop=(t == 1), perf_mode=DR)
                    # undo the host fp8 weight pre-scale (x64). Evacs alternate
                    # ACT/DVE: the DR matmuls are so cheap the stage is
                    # evac-paced, so one engine alone would bottleneck it.
                    if mi % 2 == 0:
                        nc.scalar.activation(qk8[:, 0, mb, :], qpv[:, 0:2 * NT],
                                             AF.Copy, scale=1.0 / 64.0)
                    else:
                        nc.vector.tensor_scalar_mul(qk8[:, 0, mb, :], qpv[:, 0:2 * NT],
                                                    1.0 / 64.0)
                    if mi == 2 and pending:
                        pending.pop()()  # prev pair's w1 proj fills this stage
                    if mi == 1:
                        q0.append(emit_qk(qk8, 0, 0))
                        q0.append(emit_qk(qk8, 0, 1))

                # v token-major [128(tok), 2(chunk), H, D] bf16, per window
                vs = [None, None]
                for wi in range(2):
                    wo = wi * NT
                    v_sb = wpool.tile([128, 2, H, D], bf16, tag=f"v{wi}")
                    vs[wi] = v_sb
                    for tch, tsz in ((0, 128), (1, 68)):
                        vpv = ps_ms.tile([128, 512], fp32, tag="ms", name="vpv")
                        for kc in range(4):
                            nc.tensor.matmul(
                                vpv[0:tsz, 0:CH],
                                xT[:, kc, wo + tch * 128: wo + tch * 128 + tsz],
                                wv[:, kc, :],
                                start=(kc == 0), stop=(kc == 3))
                        # split the two chunk evacs across ACT/Pool so v_sb is
                        # ready before the first AV matmul needs it and DVE
                        # stays free for the softmax stream
                        if tch == 0:
                            nc.scalar.activation(
                                v_sb[0:tsz, tch, :, :].rearrange("p h d -> p (h d)"),
                                vpv[0:tsz, 0:CH], AF.Copy)
                        else:
                            nc.vector.tensor_copy(
                                v_sb[0:tsz, tch, :, :].rearrange("p h d -> p (h d)"),
                                vpv[0:tsz, 0:CH])

                attn_r0 = attn_window(qk8, vs[0], 0, q0)
                q1 = [emit_qk(qk8, 1, 0)]
                proj_emit(2 * pair, 0, attn_r0)
                attn_r1 = attn_window(qk8, vs[1], 1, q1)
                if pair == WPC // 2 - 1:
                    proj_emit(2 * pair + 1, 1, attn_r1)
                else:
                    pending.append(
                        lambda w=2 * pair + 1, a=attn_r1: proj_emit(w, 1, a))

                xs_cur = xs_next

            while pending:
                pending.pop()()

    nc.compile()
    return nc


def _prep_ebt(rel_pos_index, rel_bias_table):
    # ebt[p, h, khi*196 + q] = exp(table[idx[q, p + 128*khi], h]) (1.0 where k pad)
    idx = np.asarray(rel_pos_index).astype(np.int64)
    table = np.asarray(rel_bias_table, dtype=np.float32)
    g = table[idx]                      # [q, k, H]
    out = np.zeros((256, H, NT), dtype=np.float32)
    out[:NT] = g.transpose(1, 2, 0)     # [k, H, q]
    out = np.exp(out)
    return np.ascontiguousarray(
        out.reshape(2, 128, H, NT).transpose(1, 2, 0, 3).reshape(128, H, 2 * NT))


def kernel(x, qkv_w, rel_bias_table, proj_w, proj_b, rel_pos_index):
    import ml_dtypes
    from concourse.bass_utils import run_bass_kernel_spmd

    if "nc" not in _CACHE:
        _CACHE["nc"] = _build()
    nc = _CACHE["nc"]

    bf16 = ml_dtypes.bfloat16
    f8 = ml_dtypes.float8_e4m3
    # host pre-transpose: [b/2, ki, ko, wi*196+q] = x[w, q, 128*ko+ki]
    xf = np.asarray(x, dtype=np.float32).reshape(B // 2, 2, NT, 4, 128)
    xf = np.ascontiguousarray(xf.transpose(0, 4, 3, 1, 2).reshape(B // 2, 128, 4, 2 * NT))
    x16 = xf.astype(bf16)
    x8 = xf.astype(f8)
    qkv_f = np.asarray(qkv_w, dtype=np.float32)
    # x64 pre-scale lifts the tiny weights out of fp8 subnormal range;
    # undone (with the softmax scale for q) in the on-device evacuation.
    wqk = np.ascontiguousarray(qkv_f[:, :2 * CH] * 64.0).astype(f8)
    wv = np.ascontiguousarray(qkv_f[:, 2 * CH:]).astype(bf16)
    ebt = _prep_ebt(rel_pos_index, rel_bias_table).astype(bf16)
    pw = np.ascontiguousarray(np.asarray(proj_w, dtype=np.float32)).astype(bf16)
    pb = np.ascontiguousarray(np.asarray(proj_b), dtype=np.float32).reshape(1, CH)

    hw = WPC // 2
    in_maps = []
    for c in range(NCORES):
        in_maps.append({
            "x": x16[c * hw:(c + 1) * hw],
            "x8": x8[c * hw:(c + 1) * hw],
            "wqk": wqk,
            "wv": wv,
            "ebt": ebt,
            "proj_w": pw,
            "proj_b": pb,
        })
    res = run_bass_kernel_spmd(nc, in_maps, core_ids=list(range(NCORES)))
    out = np.concatenate([r["y"] for r in res.results], axis=0)
    return out.astype(np.float32)


if __name__ == "__main__":
    pass
